# revision 5
# baseline (speedup 1.0000x reference)
"""Trainium2 Bass kernel for softmax(user_emb @ id_emb.T, axis=-1).

Shapes (hardcoded): user_emb [8192, 1024] f32, id_emb [8192, 1024] f32,
out [8192, 8192] f32.

Sharding: user_emb rows split across 8 NeuronCores (1024 rows each),
id_emb replicated; each core computes its [1024, 8192] score block and
row-softmax independently; outputs concatenated on axis 0.

Per-core kernel: 3-pass fp16 hi/lo split matmul (near-fp32 accuracy at
1 cycle/row per pass on the PE vs 4 cycles/row for native fp32):
    S = Uh @ Eh.T + Ul @ Eh.T + Uh @ El.T   (Ul@El term ~2^-24, dropped)
The contraction dim (d) must sit on SBUF partitions for both operands, so
U and E tiles are transposed on-chip with PE-transpose after the fp16
split. E is streamed twice (two 512-row m-blocks); the [512, 8192] f32
score block stays in SBUF where the row softmax runs fused:
reduce_max(negate) -> in-place Exp with accum_out row sums ->
reciprocal -> in-place scale -> output DMA.
"""
import numpy as np

P = 128          # partitions
D = 1024         # embedding dim (contraction)
SEQ = 8192       # id_emb rows (softmax axis)
ROWS = 1024      # user rows per core
NCORES = 8
KT = D // P      # 8 contraction chunks
NW = 512         # matmul moving free dim (one PSUM bank of f32)
NT = SEQ // NW   # 16 n-tiles
MT = ROWS // P   # 8 m-tiles per core
MB = 4           # m-tiles per E-sweep
NSWEEP = (MT + MB - 1) // MB

_CACHE = {}


def _build(reps=1):
    import concourse.tile as tile
    from concourse import bacc, mybir, masks

    F32 = mybir.dt.float32
    F16 = mybir.dt.float16
    EXP = mybir.ActivationFunctionType.Exp
    AX = mybir.AxisListType.X
    MAX = mybir.AluOpType.max

    nc = bacc.Bacc("TRN2", target_bir_lowering=False, debug=False,
                   num_devices=NCORES)
    u = nc.dram_tensor("u", [ROWS, D], F32, kind="ExternalInput").ap()
    e = nc.dram_tensor("e", [SEQ, D], F32, kind="ExternalInput").ap()
    o = nc.dram_tensor("o", [ROWS, SEQ], F32, kind="ExternalOutput").ap()

    with tile.TileContext(nc) as tc:
        with (
            tc.tile_pool(name="const", bufs=1) as constp,
            tc.tile_pool(name="ut", bufs=1) as utp,
            tc.tile_pool(name="sblk", bufs=MB) as sp,
            tc.tile_pool(name="stage", bufs=2) as stp,
            tc.tile_pool(name="split", bufs=2) as splitp,
            tc.tile_pool(name="et", bufs=2) as etp,
            tc.tile_pool(name="stats", bufs=2 * MB) as statp,
            tc.tile_pool(name="pst", bufs=4, space="PSUM") as pst,
            tc.tile_pool(name="pss", bufs=4, space="PSUM") as pss,
        ):
            ident = constp.tile([P, P], F16, tag="ident")
            masks.make_identity(nc, ident[:])

            for rep in range(reps):
                for s in range(NSWEEP):
                    m_lo = s * MB
                    m_hi = min(m_lo + MB, MT)
                    nmb = m_hi - m_lo
                    # load this sweep's U m-tiles, split fp16 hi/lo, and
                    # transpose to UT layout [P(d), KT*nmb*P] with columns
                    # k*(nmb*P) + ml*P + i.
                    ut_h = utp.tile([P, KT * MB * P], F16, tag="ut_h",
                                    name=f"ut_h_{rep}_{s}")
                    ut_l = utp.tile([P, KT * MB * P], F16, tag="ut_l",
                                    name=f"ut_l_{rep}_{s}")
                    for ml in range(nmb):
                        m = m_lo + ml
                        st = stp.tile([P, D], F32, tag="stage")
                        nc.sync.dma_start(st[:], u[m * P:(m + 1) * P, :])
                        hi = splitp.tile([P, D], F16, tag="hi")
                        nc.scalar.copy(hi[:], st[:])
                        lo = splitp.tile([P, D], F16, tag="lo")
                        nc.vector.tensor_sub(lo[:], st[:], hi[:])
                        for src, dst in ((hi, ut_h), (lo, ut_l)):
                            pt = pst.tile([P, D], F16, tag="pst")
                            for k in range(KT):
                                nc.tensor.transpose(pt[:, k * P:(k + 1) * P],
                                                    src[:, k * P:(k + 1) * P],
                                                    ident[:])
                            dview = dst[:].rearrange("p (k r) -> p k r", k=KT)
                            nc.vector.tensor_copy(
                                dview[:, :, ml * P:(ml + 1) * P],
                                pt[:].rearrange("p (k i) -> p k i", k=KT))
                    stiles = [sp.tile([P, SEQ], F32, tag="sblk",
                                      name=f"s_{rep}_{s}_{i}")
                              for i in range(m_hi - m_lo)]
                    for n in range(NT):
                        # load + split + transpose one [NW, D] slab of E
                        et_h = etp.tile([P, KT * NW], F16, tag="et_h")
                        et_l = etp.tile([P, KT * NW], F16, tag="et_l")
                        for t in range(NW // P):
                            st = stp.tile([P, D], F32, tag="stage")
                            r0 = n * NW + t * P
                            nc.sync.dma_start(st[:], e[r0:r0 + P, :])
                            hi = splitp.tile([P, D], F16, tag="hi")
                            nc.scalar.copy(hi[:], st[:])
                            lo = splitp.tile([P, D], F16, tag="lo")
                            nc.vector.tensor_sub(lo[:], st[:], hi[:])
                            for idx, (src, dst) in enumerate(
                                    ((hi, et_h), (lo, et_l))):
                                pt = pst.tile([P, D], F16, tag="pst")
                                for k in range(KT):
                                    nc.tensor.transpose(
                                        pt[:, k * P:(k + 1) * P],
                                        src[:, k * P:(k + 1) * P],
                                        ident[:])
                                dview = dst[:].rearrange(
                                    "p (k w) -> p k w", k=KT)
                                pview = pt[:].rearrange(
                                    "p (k i) -> p k i", k=KT)
                                eng = nc.vector if (t + idx) % 2 else nc.scalar
                                if eng is nc.vector:
                                    nc.vector.tensor_copy(
                                        dview[:, :, t * P:(t + 1) * P], pview)
                                else:
                                    nc.scalar.copy(
                                        dview[:, :, t * P:(t + 1) * P], pview)
                        # matmuls: 3 passes x KT chunks per m-tile
                        for ml in range(m_hi - m_lo):
                            m = m_lo + ml
                            acc = pss.tile([P, NW], F32, tag="pss")
                            n_mm = 3 * KT
                            i_mm = 0
                            for k in range(KT):
                                kb = k * MB * P
                                uh = ut_h[:, kb + ml * P:kb + (ml + 1) * P]
                                ul = ut_l[:, kb + ml * P:kb + (ml + 1) * P]
                                eh = et_h[:, k * NW:(k + 1) * NW]
                                el = et_l[:, k * NW:(k + 1) * NW]
                                for lhsT, rhs in ((uh, eh), (ul, eh),
                                                  (uh, el)):
                                    nc.tensor.matmul(
                                        acc[:], lhsT, rhs,
                                        start=(i_mm == 0),
                                        stop=(i_mm == n_mm - 1))
                                    i_mm += 1
                            dst = stiles[ml][:, n * NW:(n + 1) * NW]
                            if (n + ml) % 2:
                                nc.vector.tensor_copy(dst, acc[:])
                            else:
                                nc.scalar.copy(dst, acc[:])
                    # fused row softmax on each finished [P, SEQ] block
                    for ml in range(m_hi - m_lo):
                        m = m_lo + ml
                        stile = stiles[ml]
                        negmx = statp.tile([P, 1], F32, tag="negmx")
                        nc.vector.tensor_reduce(negmx[:], stile[:], axis=AX,
                                                op=MAX, negate=True)
                        sm = statp.tile([P, 1], F32, tag="sm")
                        nc.scalar.activation(stile[:], stile[:], EXP,
                                             bias=negmx[:], scale=1.0,
                                             accum_out=sm[:])
                        rcp = statp.tile([P, 1], F32, tag="rcp")
                        nc.vector.reciprocal(rcp[:], sm[:])
                        nc.vector.tensor_scalar_mul(stile[:], stile[:],
                                                    rcp[:])
                        nc.sync.dma_start(o[m * P:(m + 1) * P, :], stile[:])
    nc.compile()
    return nc


def _get_nc(reps=1):
    if reps not in _CACHE:
        _CACHE[reps] = _build(reps)
    return _CACHE[reps]


def kernel(user_emb: np.ndarray, id_emb: np.ndarray) -> np.ndarray:
    from concourse.bass_utils import run_bass_kernel_spmd

    nc = _get_nc()
    user_emb = np.ascontiguousarray(user_emb, dtype=np.float32)
    id_emb = np.ascontiguousarray(id_emb, dtype=np.float32)
    rows = ROWS
    in_maps = [
        {"u": user_emb[c * rows:(c + 1) * rows], "e": id_emb}
        for c in range(NCORES)
    ]
    res = run_bass_kernel_spmd(nc, in_maps, list(range(NCORES)))
    return np.concatenate([res.results[c]["o"] for c in range(NCORES)],
                          axis=0)
